# revision 1
# baseline (speedup 1.0000x reference)
"""Multi-head attention TRN2 kernel: 8-core head-sharded tensor parallelism.

Full inputs in, full output out. Each core computes 2 of the 16 heads:
QKV projection (its column slice), flash-style attention, and a partial
out-projection against its row slice of Wo. Host sums the 8 partials and
adds (bv @ Wo + bo) once; the K bias is dropped entirely (it only adds a
per-query constant to the logits, which softmax cancels).

Per-core device program (identical SPMD; per-core weight slices via in_maps):
  projections: Q^T/K^T [128, T] (weights stationary) and V^T (x stationary,
           Wv moving) computed in 256-token units that are just-in-time
           interleaved into the attention loop's PE slack, so the scalar
           engine (exp) starts almost immediately and stays busy
  attention: per (batch, q-block, key-tile): both heads' score matmuls are
           row-tiled (partitions 0-63 / 64-127) into one 2-bank PSUM tile,
           one 1024-wide exp on ACT, ctx accumulation on PE against a
           ones-augmented V (the ones column emits softmax row-sums free)
  norm:    reciprocal of the row-sums, one broadcast matmul per head,
           multiply into ctx2t — split into DVE/PE work items so neither
           engine FIFO head-blocks
  out-proj: y[t, fo] = ctx2t-token-tile-stationary @ Wo_slice, interleaved,
           bf16 partials to DRAM

All SBUF tensors bf16 (fp32 PSUM accumulation).
"""
import sys

sys.path.insert(0, "/opt/trn_rl_repo")

from collections import deque
from contextlib import ExitStack

import numpy as np

import concourse.bass as bass
import concourse.tile as tile
from concourse import bacc, mybir
from concourse.bass_utils import run_bass_kernel_spmd

f32 = mybir.dt.float32
bf16 = mybir.dt.bfloat16
EXP = mybir.ActivationFunctionType.Exp

N_CORES = 8
B, S, F = 2, 2048, 1024
H = 16                 # heads total
DK = F // H            # 64
HPC = H // N_CORES     # 2 heads per core
CF = HPC * DK          # 128 = per-core slice of features
T = B * S              # 4096 tokens
TU = 512               # tokens per projection unit
NU = T // TU           # 8 projection units
NKT = S // 128         # 16 key tiles per sequence
NQB = S // 512         # 4 q-blocks per sequence
NC = F // 128          # 8 contraction chunks


def build_program():
    nc = bacc.Bacc("TRN2", target_bir_lowering=False, debug=False,
                   num_devices=N_CORES)

    xt_d = nc.dram_tensor("xT", [F, T], bf16, kind="ExternalInput").ap()
    wq_d = nc.dram_tensor("Wq", [F, CF], bf16, kind="ExternalInput").ap()
    wk_d = nc.dram_tensor("Wk", [F, CF], bf16, kind="ExternalInput").ap()
    wv_d = nc.dram_tensor("Wv", [F, CF], bf16, kind="ExternalInput").ap()
    bq_d = nc.dram_tensor("bq", [CF, 1], f32, kind="ExternalInput").ap()
    wo_d = nc.dram_tensor("Wo", [CF, F], bf16, kind="ExternalInput").ap()
    yp_d = nc.dram_tensor("yp", [T, F], bf16, kind="ExternalOutput").ap()

    with tile.TileContext(nc) as tc, ExitStack() as ctx:
        const = ctx.enter_context(tc.tile_pool(name="const", bufs=1))
        big = ctx.enter_context(tc.tile_pool(name="big", bufs=1))
        etp = ctx.enter_context(tc.tile_pool(name="etp", bufs=6))
        small = ctx.enter_context(tc.tile_pool(name="small", bufs=4))
        ypool = ctx.enter_context(tc.tile_pool(name="ypool", bufs=4))

        # ---- constants / inputs ----
        wq_sb = const.tile([128, NC, CF], bf16)
        nc.sync.dma_start(wq_sb, wq_d.rearrange("(a p) n -> p a n", p=128))
        # x input, one DMA per 512-token unit so unit j only waits its slice
        xall = const.tile([128, NU, NC, TU], bf16)
        nc.sync.dma_start(
            xall[:, 0, :, :],
            xt_d[:, 0:TU].rearrange("(a p) t -> p a t", p=128))
        wk_sb = const.tile([128, NC, CF], bf16)
        nc.sync.dma_start(wk_sb, wk_d.rearrange("(a p) n -> p a n", p=128))
        wv_sb = const.tile([128, NC, CF], bf16)
        nc.sync.dma_start(wv_sb, wv_d.rearrange("(a p) n -> p a n", p=128))
        bq_sb = const.tile([128, 1], f32)
        nc.sync.dma_start(bq_sb, bq_d)
        wo_sb = const.tile([128, F], bf16)
        nc.sync.dma_start(wo_sb, wo_d)
        for j in range(1, NU):
            nc.sync.dma_start(
                xall[:, j, :, :],
                xt_d[:, j * TU:(j + 1) * TU]
                .rearrange("(a p) t -> p a t", p=128))
        # ones row for the rowsum broadcast matmul
        ones_bf = const.tile([1, 64], bf16)
        nc.vector.memset(ones_bf, 1.0)

        # ---- persistent activations (all bf16) ----
        qt_sb = big.tile([128, T], bf16)       # [2 heads x 64 d, tokens]
        kt_sb = big.tile([128, T], bf16)
        # vaug per (b,kt): [tok-in-ktile, 130]:
        #   cols 0:64 = V head0, col 64 = ones, cols 65:129 = V head1,
        #   col 129 = ones.  head h ctx stationary = cols h*65 : h*65+65.
        vaug_sb = big.tile([128, B, NKT, 130], bf16)
        ctx2t_sb = big.tile([128, B, S], bf16)  # [2 heads x 64 d, b, tokens]

        nc.vector.memset(vaug_sb[:, :, :, 64:65], 1.0)
        nc.vector.memset(vaug_sb[:, :, :, 129:130], 1.0)

        # ---- PSUM pools ----
        sc_ps = ctx.enter_context(
            tc.tile_pool(name="sc_ps", bufs=2, space="PSUM"))   # 4 banks
        pc_ps = ctx.enter_context(
            tc.tile_pool(name="pc_ps", bufs=3, space="PSUM"))   # 3 banks
        aux_ps = ctx.enter_context(
            tc.tile_pool(name="aux_ps", bufs=1, space="PSUM"))  # 1 bank

        # ---- projection units (Q, K, V^T per 512 tokens) ----
        def emit_aq(j, pool, tag):
            """Q projection for tokens [j*512, (j+1)*512)."""
            pq = pool.tile([128, TU], f32, tag=tag, name=f"pq{j}")
            for c in range(NC):
                nc.tensor.matmul(pq, wq_sb[:, c, :], xall[:, j, c, :],
                                 start=(c == 0), stop=(c == NC - 1))
            sl = slice(j * TU, (j + 1) * TU)
            nc.vector.tensor_scalar_add(qt_sb[:, sl], pq, bq_sb)

        def emit_ak(j, pool, tag):
            """K projection for tokens [j*512, (j+1)*512)."""
            pk = pool.tile([128, TU], f32, tag=tag, name=f"pk{j}")
            for c in range(NC):
                nc.tensor.matmul(pk, wk_sb[:, c, :], xall[:, j, c, :],
                                 start=(c == 0), stop=(c == NC - 1))
            sl = slice(j * TU, (j + 1) * TU)
            nc.vector.tensor_copy(kt_sb[:, sl], pk)

        def emit_av(j, pool, tag):
            """V^T for the 4 key tiles in tokens [j*512, (j+1)*512)."""
            pv = pool.tile([128, 4, 128], f32, tag=tag, name=f"pv{j}")
            for i in range(4):
                for c in range(NC):
                    nc.tensor.matmul(
                        pv[:, i, :],
                        xall[:, j, c, i * 128:(i + 1) * 128],
                        wv_sb[:, c, :],
                        start=(i == 0 and c == 0),
                        stop=(i == 3 and c == NC - 1))
            bj = j // 4
            kt0 = (j % 4) * 4
            nc.vector.tensor_copy(
                vaug_sb[:, bj, kt0:kt0 + 4, 0:64], pv[:, :, 0:64])
            nc.vector.tensor_copy(
                vaug_sb[:, bj, kt0:kt0 + 4, 65:129], pv[:, :, 64:128])

        # serial head: just enough projections for the first attention steps,
        # run in parallel across the (still empty) pc ring
        emit_aq(0, pc_ps, "pc")
        emit_ak(0, pc_ps, "pc")
        emit_av(0, pc_ps, "pc")

        # remaining units, just-in-time: (deadline_iteration, kind, j)
        # b0 tight (data-dependent), b1 spread gently through b0's steps
        jit = deque(sorted([
            (2, "k", 1), (4, "v", 1), (6, "k", 2), (8, "v", 2),
            (10, "k", 3), (12, "v", 3), (14, "q", 1),
            (16, "q", 4), (18, "k", 4), (20, "v", 4), (22, "q", 5),
            (24, "k", 5), (26, "v", 5), (28, "q", 2), (30, "k", 6),
            (32, "v", 6), (34, "q", 6), (36, "k", 7), (38, "v", 7),
            (40, "q", 3), (42, "q", 7),
        ], key=lambda t: t[0]))

        # ---- attention steps: one per (b, qb, kt) ----
        steps = []
        for b in range(B):
            for qb in range(NQB):
                for kt in range(NKT):
                    steps.append((b, qb, kt))

        score_ps = {}
        exp_sb = {}
        ctx_ps = {}
        work = deque()   # deferred norm / out-proj units

        def emit_scores(i):
            b, qb, kt = steps[i]
            pss = sc_ps.tile([128, 2, 512], f32, tag="sc", name=f"sc{i}")
            for h in range(2):
                nc.tensor.matmul(
                    pss[:, h, :],
                    kt_sb[h * 64:(h + 1) * 64,
                          b * S + kt * 128:b * S + (kt + 1) * 128],
                    qt_sb[h * 64:(h + 1) * 64,
                          b * S + qb * 512:b * S + (qb + 1) * 512],
                    start=True, stop=True)
            score_ps[i] = pss

        def emit_exp(i):
            et = etp.tile([128, 2, 512], bf16, tag="et", name=f"et{i}")
            nc.scalar.activation(et, score_ps.pop(i), EXP)
            exp_sb[i] = et

        def emit_ctx(i):
            b, qb, kt = steps[i]
            if kt == 0:
                ctx_ps[(b, qb, 0)] = pc_ps.tile(
                    [65, 512], f32, tag="pc", name=f"pc{i}h0")
                ctx_ps[(b, qb, 1)] = pc_ps.tile(
                    [65, 512], f32, tag="pc", name=f"pc{i}h1")
            et = exp_sb.pop(i)
            for h in range(2):
                nc.tensor.matmul(
                    ctx_ps[(b, qb, h)],
                    vaug_sb[:, b, kt, h * 65:h * 65 + 65],
                    et[:, h, :],
                    start=(kt == 0), stop=(kt == NKT - 1))
            if kt == NKT - 1:
                work.append(("norm_dve", b, qb))

        norm_state = {}

        def emit_norm_dve(b, qb):
            """DVE half: rowsum copies + reciprocal + bf16 cast."""
            pc0 = ctx_ps[(b, qb, 0)]
            pc1 = ctx_ps[(b, qb, 1)]
            rs = small.tile([1, 2, 512], f32, tag="rs", name=f"rs{b}{qb}")
            # plain copies: custom-DVE ops don't partition-shift their input
            nc.vector.tensor_copy(rs[0:1, 0, :], pc0[64:65, :])
            nc.vector.tensor_copy(rs[0:1, 1, :], pc1[64:65, :])
            rcp = small.tile([1, 2, 512], f32, tag="rcp", name=f"rcp{b}{qb}")
            nc.vector.reciprocal_approx_fast(rcp, rs)
            rcpb = small.tile([1, 2, 512], bf16, tag="rcpb",
                              name=f"rcpb{b}{qb}")
            nc.vector.tensor_copy(rcpb, rcp)
            norm_state[(b, qb)] = rcpb
            work.append(("norm_pe", b, qb))

        def emit_norm_pe(b, qb):
            """PE half: broadcast matmuls + normalize into ctx2t."""
            pc0 = ctx_ps.pop((b, qb, 0))
            pc1 = ctx_ps.pop((b, qb, 1))
            rcpb = norm_state.pop((b, qb))
            dst = ctx2t_sb[:, b, qb * 512:(qb + 1) * 512]
            pcs = (pc0, pc1)
            for h in range(2):
                pb = aux_ps.tile([64, 512], f32, tag="aux",
                                 name=f"pb{b}{qb}{h}")
                nc.tensor.matmul(pb, ones_bf, rcpb[0:1, h, :],
                                 start=True, stop=True)
                pbs = small.tile([64, 512], bf16, tag="pbs",
                                 name=f"pbs{b}{qb}{h}")
                nc.vector.tensor_copy(pbs, pb)
                nc.vector.tensor_mul(dst[h * 64:(h + 1) * 64, :],
                                     pcs[h][0:64, :], pbs)
            for tt in range(4):
                work.append(("op", b, qb, tt))

        def emit_outproj(b, qb, tt):
            tok0 = qb * 512 + tt * 128
            ysb = ypool.tile([128, 1024], bf16, tag="ysb",
                             name=f"ysb{b}{qb}{tt}")
            for wh in range(2):
                py = aux_ps.tile([128, 512], f32, tag="aux",
                                 name=f"py{b}{qb}{tt}{wh}")
                nc.tensor.matmul(
                    py, ctx2t_sb[:, b, tok0:tok0 + 128],
                    wo_sb[:, wh * 512:(wh + 1) * 512],
                    start=True, stop=True)
                nc.vector.tensor_copy(ysb[:, wh * 512:(wh + 1) * 512], py)
            nc.sync.dma_start(
                yp_d[b * S + tok0:b * S + tok0 + 128, :], ysb)

        def drain_work(n=1):
            for _ in range(n):
                if not work:
                    return
                item = work.popleft()
                if item[0] == "norm_dve":
                    emit_norm_dve(item[1], item[2])
                elif item[0] == "norm_pe":
                    emit_norm_pe(item[1], item[2])
                else:
                    emit_outproj(item[1], item[2], item[3])

        def drain_jit(i):
            while jit and jit[0][0] <= i:
                _, kind, j = jit.popleft()
                if kind == "q":
                    emit_aq(j, aux_ps, "aux")
                elif kind == "k":
                    emit_ak(j, aux_ps, "aux")
                else:
                    emit_av(j, aux_ps, "aux")

        emit_scores(0)
        emit_scores(1)
        emit_exp(0)
        for i in range(2, len(steps)):
            drain_jit(i)
            emit_scores(i)
            emit_exp(i - 1)
            emit_ctx(i - 2)
            if i >= 110:
                drain_work(1)
            elif i % 3 == 0:
                drain_work(1)
        emit_exp(len(steps) - 1)
        emit_ctx(len(steps) - 2)
        emit_ctx(len(steps) - 1)
        while work:
            drain_work(1)

    nc.compile()
    return nc


_NC = None


def _to_bf16(a):
    import ml_dtypes
    return np.asarray(a, dtype=np.float32).astype(ml_dtypes.bfloat16)


def make_in_maps(inputs):
    """Build the 8 per-core input maps from full-precision inputs."""
    x = np.asarray(inputs["x"], dtype=np.float32)
    sc = 1.0 / np.sqrt(np.float32(DK))
    xT = np.ascontiguousarray(x.reshape(T, F).T)
    xT16 = _to_bf16(xT)
    in_maps = []
    for c in range(N_CORES):
        sl = slice(c * CF, (c + 1) * CF)
        in_maps.append({
            "xT": xT16,
            "Wq": _to_bf16(np.asarray(inputs["Wq"])[:, sl] * sc),
            "Wk": _to_bf16(np.asarray(inputs["Wk"])[:, sl]),
            "Wv": _to_bf16(np.asarray(inputs["Wv"])[:, sl]),
            "bq": np.ascontiguousarray(
                (np.asarray(inputs["bq"])[sl] * sc)
                .astype(np.float32).reshape(CF, 1)),
            "Wo": _to_bf16(np.asarray(inputs["Wo"])[sl, :]),
        })
    return in_maps


def combine_outputs(results, inputs):
    """Sum per-core bf16 partials, add host-side bias terms."""
    y = np.zeros((T, F), dtype=np.float64)
    for c in range(N_CORES):
        y += np.asarray(results[c]["yp"], dtype=np.float64)
    bo = np.asarray(inputs["bo"], dtype=np.float64)
    bv = np.asarray(inputs["bv"], dtype=np.float64)
    Wo = np.asarray(inputs["Wo"], dtype=np.float64)
    y += bo + bv @ Wo
    return y.astype(np.float32).reshape(B, S, F)


def kernel(x, Wq, bq, Wk, bk, Wv, bv, Wo, bo):
    global _NC
    if _NC is None:
        _NC = build_program()
    inputs = {"x": x, "Wq": Wq, "bq": bq, "Wk": Wk, "bk": bk,
              "Wv": Wv, "bv": bv, "Wo": Wo, "bo": bo}
    in_maps = make_in_maps(inputs)
    res = run_bass_kernel_spmd(_NC, in_maps, list(range(N_CORES)))
    return combine_outputs(res.results, inputs)



# revision 4
# speedup vs baseline: 1.0519x; 1.0519x over previous
"""Multi-head attention TRN2 kernel: 8-core head-sharded tensor parallelism.

Full inputs in, full output out. Each core computes 2 of the 16 heads:
QKV projection (its column slice), flash-style attention, and a partial
out-projection against its row slice of Wo. Host sums the 8 partials and
adds (bv @ Wo + bo) once; the K bias is dropped entirely (it only adds a
per-query constant to the logits, which softmax cancels).

v2 schedule (all bf16): the scalar engine's exp is the per-step floor
(~1.05 us per (b,qb,kt) step); the PE must never starve it. All PE work
is sliced into ~450 ns units (8 matmuls of N=128, the granularity at
which the PE pipelines at 55 ns/matmul) and emitted by a budget
scheduler between score pairs. Score pairs for the two heads run
concurrently on disjoint PE row groups (tile_position inferred from
base partitions). ctx lags scores by 4 steps and is drained
PSUM->SBUF immediately at the last key tile so the 2-buf ctx pool
never blocks the next q-block. Projections stream just-in-time with
slice-level deadlines.
"""
import sys

sys.path.insert(0, "/opt/trn_rl_repo")

from collections import deque
from contextlib import ExitStack

import numpy as np

import concourse.bass as bass
import concourse.tile as tile
from concourse import bacc, mybir
from concourse.bass_utils import run_bass_kernel_spmd

f32 = mybir.dt.float32
bf16 = mybir.dt.bfloat16
EXP = mybir.ActivationFunctionType.Exp

N_CORES = 8
B, S, F = 2, 2048, 1024
H = 16                 # heads total
DK = F // H            # 64
HPC = H // N_CORES     # 2 heads per core
CF = HPC * DK          # 128 = per-core slice of features
T = B * S              # 4096 tokens
TU = 512               # tokens per projection unit
NU = T // TU           # 8 projection units
NKT = S // 128         # 16 key tiles per sequence
NQB = S // 512         # 4 q-blocks per sequence
NC = F // 128          # 8 contraction chunks
CTX_LAG = 4


def build_program():
    nc = bacc.Bacc("TRN2", target_bir_lowering=False, debug=False,
                   num_devices=N_CORES)

    xt_d = nc.dram_tensor("xT", [F, T], bf16, kind="ExternalInput").ap()
    wq_d = nc.dram_tensor("Wq", [F, CF], bf16, kind="ExternalInput").ap()
    wk_d = nc.dram_tensor("Wk", [F, CF], bf16, kind="ExternalInput").ap()
    wv_d = nc.dram_tensor("Wv", [F, CF], bf16, kind="ExternalInput").ap()
    bq_d = nc.dram_tensor("bq", [CF, 1], f32, kind="ExternalInput").ap()
    wo_d = nc.dram_tensor("Wo", [CF, F], bf16, kind="ExternalInput").ap()
    yp_d = nc.dram_tensor("yp", [T, F], bf16, kind="ExternalOutput").ap()

    with tile.TileContext(nc) as tc, ExitStack() as ctx:
        const = ctx.enter_context(tc.tile_pool(name="const", bufs=1))
        big = ctx.enter_context(tc.tile_pool(name="big", bufs=1))
        etp = ctx.enter_context(tc.tile_pool(name="etp", bufs=6))
        csb = ctx.enter_context(tc.tile_pool(name="csb", bufs=2))
        small = ctx.enter_context(tc.tile_pool(name="small", bufs=4))
        ypool = ctx.enter_context(tc.tile_pool(name="ypool", bufs=4))

        # ---- inputs: DMA order puts the step-0 critical path first ----
        xall = const.tile([128, NU, NC, TU], bf16)
        wq_sb = const.tile([128, NC, CF], bf16)
        wk_sb = const.tile([128, NC, CF], bf16)
        wv_sb = const.tile([128, NC, CF], bf16)
        bq_sb = const.tile([128, 1], f32)
        wo_sb = const.tile([128, F], bf16)

        def dma_x(j):
            nc.sync.dma_start(
                xall[:, j, :, :],
                xt_d[:, j * TU:(j + 1) * TU]
                .rearrange("(a p) t -> p a t", p=128))

        dma_x(0)
        nc.sync.dma_start(wq_sb, wq_d.rearrange("(a p) n -> p a n", p=128))
        nc.sync.dma_start(wk_sb, wk_d.rearrange("(a p) n -> p a n", p=128))
        nc.sync.dma_start(bq_sb, bq_d)
        dma_x(1)
        nc.sync.dma_start(wv_sb, wv_d.rearrange("(a p) n -> p a n", p=128))
        dma_x(2)
        nc.sync.dma_start(wo_sb, wo_d)
        for j in range(3, NU):
            dma_x(j)

        ones_bf = const.tile([1, 64], bf16)
        nc.vector.memset(ones_bf, 1.0)

        # ---- persistent activations (all bf16) ----
        qt_sb = big.tile([128, T], bf16)       # [2 heads x 64 d, tokens]
        kt_sb = big.tile([128, T], bf16)
        # vaug per (b,kt): [tok-in-ktile, 130]:
        #   cols 0:64 = V head0, col 64 = ones, cols 65:129 = V head1,
        #   col 129 = ones.  head h ctx stationary = cols h*65 : h*65+65.
        vaug_sb = big.tile([128, B, NKT, 130], bf16)
        ctx2t_sb = big.tile([128, B, S], bf16)  # [2 heads x 64 d, b, tokens]

        nc.vector.memset(vaug_sb[:, :, :, 64:65], 1.0)
        nc.vector.memset(vaug_sb[:, :, :, 129:130], 1.0)

        # ---- PSUM pools: 4 + 2 + 2 banks ----
        sc_ps = ctx.enter_context(
            tc.tile_pool(name="sc_ps", bufs=2, space="PSUM"))   # 4 banks
        pc_ps = ctx.enter_context(
            tc.tile_pool(name="pc_ps", bufs=2, space="PSUM"))   # 2 banks
        aux_ps = ctx.enter_context(
            tc.tile_pool(name="aux_ps", bufs=2, space="PSUM"))  # 2 banks

        # ---- projection slices: 8 matmuls of N=128 + a DVE drain ----
        def emit_q_slice(j, q):
            tq = j * TU + q * 128
            pq = aux_ps.tile([128, 128], f32, tag="aux", name=f"pq{j}_{q}")
            for c in range(NC):
                nc.tensor.matmul(pq, wq_sb[:, c, :],
                                 xall[:, j, c, q * 128:(q + 1) * 128],
                                 start=(c == 0), stop=(c == NC - 1))
            nc.vector.tensor_scalar_add(qt_sb[:, tq:tq + 128], pq, bq_sb)

        def emit_k_slice(j, q):
            tq = j * TU + q * 128
            pk = aux_ps.tile([128, 128], f32, tag="aux", name=f"pk{j}_{q}")
            for c in range(NC):
                nc.tensor.matmul(pk, wk_sb[:, c, :],
                                 xall[:, j, c, q * 128:(q + 1) * 128],
                                 start=(c == 0), stop=(c == NC - 1))
            nc.vector.tensor_copy(kt_sb[:, tq:tq + 128], pk)

        def emit_v_slice(j, q):
            pv = aux_ps.tile([128, 128], f32, tag="aux", name=f"pv{j}_{q}")
            for c in range(NC):
                nc.tensor.matmul(pv,
                                 xall[:, j, c, q * 128:(q + 1) * 128],
                                 wv_sb[:, c, :],
                                 start=(c == 0), stop=(c == NC - 1))
            b = j // (NU // B)
            kt = (j % (NU // B)) * 4 + q
            nc.vector.tensor_copy(vaug_sb[:, b, kt, 0:64], pv[:, 0:64])
            nc.vector.tensor_copy(vaug_sb[:, b, kt, 65:129], pv[:, 64:128])

        # ---- attention steps: one per (b, qb, kt) ----
        steps = []
        for b in range(B):
            for qb in range(NQB):
                for kt in range(NKT):
                    steps.append((b, qb, kt))

        score_ps = {}
        exp_sb = {}
        ctx_ps = {}
        ctx_sb = {}

        def emit_scores(i):
            b, qb, kt = steps[i]
            pss = sc_ps.tile([128, 2, 512], f32, tag="sc", name=f"sc{i}")
            for h in range(2):
                nc.tensor.matmul(
                    pss[:, h, :],
                    kt_sb[h * 64:(h + 1) * 64,
                          b * S + kt * 128:b * S + (kt + 1) * 128],
                    qt_sb[h * 64:(h + 1) * 64,
                          b * S + qb * 512:b * S + (qb + 1) * 512],
                    start=True, stop=True)
            score_ps[i] = pss

        def emit_exp(i):
            et = etp.tile([128, 2, 512], bf16, tag="et", name=f"et{i}")
            nc.scalar.activation(et, score_ps.pop(i), EXP)
            exp_sb[i] = et

        workq = deque()   # dynamic drain items: norm / outproj

        def emit_ctx(i):
            b, qb, kt = steps[i]
            if kt == 0:
                ctx_ps[(b, qb, 0)] = pc_ps.tile(
                    [65, 512], f32, tag="pc", name=f"pc{i}h0")
                ctx_ps[(b, qb, 1)] = pc_ps.tile(
                    [65, 512], f32, tag="pc", name=f"pc{i}h1")
            et = exp_sb.pop(i)
            for h in range(2):
                nc.tensor.matmul(
                    ctx_ps[(b, qb, h)],
                    vaug_sb[:, b, kt, h * 65:h * 65 + 65],
                    et[:, h, :],
                    start=(kt == 0), stop=(kt == NKT - 1))
            if kt == NKT - 1:
                # drain ctx psum to SBUF immediately: frees both pc bufs
                # before the next q-block's first ctx matmul needs them
                cs = csb.tile([65, 2, 512], bf16, tag="cs",
                              name=f"cs{b}{qb}")
                nc.vector.tensor_copy(cs[:, 0, :], ctx_ps.pop((b, qb, 0)))
                nc.vector.tensor_copy(cs[:, 1, :], ctx_ps.pop((b, qb, 1)))
                ctx_sb[(b, qb)] = cs
                workq.append(("norm", b, qb))

        def emit_norm(b, qb):
            """rcp of the rowsums + broadcast matmul + normalize ctx2t."""
            cs = ctx_sb[(b, qb)]
            rs = small.tile([1, 2, 512], f32, tag="rs", name=f"rs{b}{qb}")
            nc.vector.tensor_copy(rs, cs[64:65, :, :])
            rcp = small.tile([1, 2, 512], f32, tag="rcp",
                             name=f"rcp{b}{qb}")
            nc.vector.reciprocal_approx_fast(rcp, rs)
            rcpb = small.tile([1, 2, 512], bf16, tag="rcpb",
                              name=f"rcpb{b}{qb}")
            nc.vector.tensor_copy(rcpb, rcp)
            dst = ctx2t_sb[:, b, qb * 512:(qb + 1) * 512]
            for h in range(2):
                pb = aux_ps.tile([64, 512], f32, tag="aux",
                                 name=f"pb{b}{qb}{h}")
                nc.tensor.matmul(pb, ones_bf, rcpb[0:1, h, :],
                                 start=True, stop=True)
                pbs = small.tile([64, 512], bf16, tag="pbs",
                                 name=f"pbs{b}{qb}{h}")
                nc.vector.tensor_copy(pbs, pb)
                nc.vector.tensor_mul(dst[h * 64:(h + 1) * 64, :],
                                     cs[0:64, h, :], pbs)
            for tt in range(4):
                workq.append(("op", b, qb, tt))

        def emit_outproj(b, qb, tt):
            tok0 = qb * 512 + tt * 128
            ysb = ypool.tile([128, 1024], bf16, tag="ysb",
                             name=f"ysb{b}{qb}{tt}")
            for wh in range(2):
                py = aux_ps.tile([128, 512], f32, tag="aux",
                                 name=f"py{b}{qb}{tt}{wh}")
                nc.tensor.matmul(
                    py, ctx2t_sb[:, b, tok0:tok0 + 128],
                    wo_sb[:, wh * 512:(wh + 1) * 512],
                    start=True, stop=True)
                nc.vector.tensor_copy(ysb[:, wh * 512:(wh + 1) * 512], py)
            nc.sync.dma_start(
                yp_d[b * S + tok0:b * S + tok0 + 128, :], ysb)

        # ---- background work queue: slice-level deadlines ----
        # (deadline, kind, j, q); kind in {"q","k","v"}
        bgq = []
        for u in range(1, NU):
            base = 0 if u < NU // B else 64
            uu = u % (NU // B)
            for q in range(4):
                kt = uu * 4 + q
                bgq.append((base + kt - 2, "k", u, q))
                bgq.append((base + kt + CTX_LAG - 2, "v", u, q))
                qb_step = base + 16 * uu
                bgq.append((qb_step - 5 + q, "q", u, q))
        bgq.sort(key=lambda t: t[0])
        bgq = deque(bgq)

        SLICE_COST = 450
        WORK_COST = {"norm": 500, "op": 500}
        BG_TOTAL = (len(bgq) * SLICE_COST
                    + 8 * WORK_COST["norm"] + 32 * WORK_COST["op"])
        RATE = BG_TOTAL / 126.0
        spent = [0.0]

        def emit_bg_slice():
            _, kind, j, q = bgq.popleft()
            if kind == "q":
                emit_q_slice(j, q)
            elif kind == "k":
                emit_k_slice(j, q)
            else:
                emit_v_slice(j, q)
            spent[0] += SLICE_COST

        def emit_work_item():
            item = workq.popleft()
            if item[0] == "norm":
                emit_norm(item[1], item[2])
            else:
                emit_outproj(item[1], item[2], item[3])
            spent[0] += WORK_COST[item[0]]

        def drain_background(i, cap=1400):
            # hard deadlines first, regardless of budget
            while bgq and bgq[0][0] <= i:
                emit_bg_slice()
            start = spent[0]
            target = (i + 1) * RATE
            while spent[0] < target and spent[0] - start < cap:
                # norm items release csb/pc resources: highest priority
                if workq and workq[0][0] == "norm":
                    emit_work_item()
                elif bgq and bgq[0][0] <= i + 4:
                    emit_bg_slice()
                elif workq:
                    emit_work_item()
                elif bgq:
                    emit_bg_slice()
                else:
                    break

        # ---- prologue: minimal path to the first exp ----
        emit_k_slice(0, 0)
        for q in range(4):
            emit_q_slice(0, q)
        emit_scores(0)
        emit_k_slice(0, 1)
        emit_scores(1)
        emit_exp(0)
        emit_k_slice(0, 2)
        emit_k_slice(0, 3)
        for q in range(4):
            emit_v_slice(0, q)

        # ---- main loop ----
        for i in range(2, len(steps)):
            emit_scores(i)
            emit_exp(i - 1)
            if i >= CTX_LAG:
                emit_ctx(i - CTX_LAG)
            drain_background(i)
        emit_exp(len(steps) - 1)
        for i in range(len(steps) - CTX_LAG, len(steps)):
            emit_ctx(i)
        while workq or bgq:
            if bgq:
                emit_bg_slice()
            else:
                emit_work_item()

    nc.compile()
    return nc


_NC = None


def _to_bf16(a):
    import ml_dtypes
    return np.asarray(a, dtype=np.float32).astype(ml_dtypes.bfloat16)


def make_in_maps(inputs):
    """Build the 8 per-core input maps from full-precision inputs."""
    x = np.asarray(inputs["x"], dtype=np.float32)
    sc = 1.0 / np.sqrt(np.float32(DK))
    xT = np.ascontiguousarray(x.reshape(T, F).T)
    xT16 = _to_bf16(xT)
    in_maps = []
    for c in range(N_CORES):
        sl = slice(c * CF, (c + 1) * CF)
        in_maps.append({
            "xT": xT16,
            "Wq": _to_bf16(np.asarray(inputs["Wq"])[:, sl] * sc),
            "Wk": _to_bf16(np.asarray(inputs["Wk"])[:, sl]),
            "Wv": _to_bf16(np.asarray(inputs["Wv"])[:, sl]),
            "bq": np.ascontiguousarray(
                (np.asarray(inputs["bq"])[sl] * sc)
                .astype(np.float32).reshape(CF, 1)),
            "Wo": _to_bf16(np.asarray(inputs["Wo"])[sl, :]),
        })
    return in_maps


def combine_outputs(results, inputs):
    """Sum per-core bf16 partials, add host-side bias terms."""
    y = np.zeros((T, F), dtype=np.float64)
    for c in range(N_CORES):
        y += np.asarray(results[c]["yp"], dtype=np.float64)
    bo = np.asarray(inputs["bo"], dtype=np.float64)
    bv = np.asarray(inputs["bv"], dtype=np.float64)
    Wo = np.asarray(inputs["Wo"], dtype=np.float64)
    y += bo + bv @ Wo
    return y.astype(np.float32).reshape(B, S, F)


def kernel(x, Wq, bq, Wk, bk, Wv, bv, Wo, bo):
    global _NC
    if _NC is None:
        _NC = build_program()
    inputs = {"x": x, "Wq": Wq, "bq": bq, "Wk": Wk, "bk": bk,
              "Wv": Wv, "bv": bv, "Wo": Wo, "bo": bo}
    in_maps = make_in_maps(inputs)
    res = run_bass_kernel_spmd(_NC, in_maps, list(range(N_CORES)))
    return combine_outputs(res.results, inputs)


# revision 6
# speedup vs baseline: 1.0626x; 1.0101x over previous
"""Multi-head attention TRN2 kernel: 8-core head-sharded tensor parallelism.

Full inputs in, full output out. Each core computes 2 of the 16 heads:
QKV projection (its column slice), flash-style attention, and a partial
out-projection against its row slice of Wo. Host sums the 8 partials and
adds (bv @ Wo + bo) once; the K bias is dropped entirely (it only adds a
per-query constant to the logits, which softmax cancels).

v3 schedule (all bf16): the scalar engine's exp (~1.05-1.15 us per
(b,qb,kt) step) is the pacing floor; the PE must keep pace without
starving it. Score pairs for the two heads run concurrently on disjoint
PE row groups. ctx stationaries are zero-padded to 128 columns so FWL
(fast weight load) hides their LDWEIGHTS; garbage output rows 65-127
are ignored. Projections stream as half-unit slices (~1 us of PE)
scheduled by budget between steps; norm is split into a DVE item and a
PE item so the reciprocal chain never head-of-line blocks the PE queue.
ctx lags scores by 4 steps and is drained PSUM->SBUF at the last key
tile so the 2-buf ctx pool frees before the next q-block needs it.
"""
import sys

sys.path.insert(0, "/opt/trn_rl_repo")

from collections import deque
from contextlib import ExitStack

import numpy as np

import concourse.bass as bass
import concourse.tile as tile
from concourse import bacc, mybir
from concourse.bass_utils import run_bass_kernel_spmd

f32 = mybir.dt.float32
bf16 = mybir.dt.bfloat16
EXP = mybir.ActivationFunctionType.Exp

N_CORES = 8
B, S, F = 2, 2048, 1024
H = 16                 # heads total
DK = F // H            # 64
HPC = H // N_CORES     # 2 heads per core
CF = HPC * DK          # 128 = per-core slice of features
T = B * S              # 4096 tokens
TU = 512               # tokens per projection unit
NU = T // TU           # 8 projection units
NKT = S // 128         # 16 key tiles per sequence
NQB = S // 512         # 4 q-blocks per sequence
NC = F // 128          # 8 contraction chunks
CTX_LAG = 4


def build_program():
    nc = bacc.Bacc("TRN2", target_bir_lowering=False, debug=False,
                   num_devices=N_CORES)

    xt_d = nc.dram_tensor("xT", [F, T], bf16, kind="ExternalInput").ap()
    wq_d = nc.dram_tensor("Wq", [F, CF], bf16, kind="ExternalInput").ap()
    wk_d = nc.dram_tensor("Wk", [F, CF], bf16, kind="ExternalInput").ap()
    wv_d = nc.dram_tensor("Wv", [F, CF], bf16, kind="ExternalInput").ap()
    bq_d = nc.dram_tensor("bq", [CF, 1], f32, kind="ExternalInput").ap()
    wo_d = nc.dram_tensor("Wo", [CF, F], bf16, kind="ExternalInput").ap()
    yp_d = nc.dram_tensor("yp", [T, F], bf16, kind="ExternalOutput").ap()

    with tile.TileContext(nc) as tc, ExitStack() as ctx:
        const = ctx.enter_context(tc.tile_pool(name="const", bufs=1))
        big = ctx.enter_context(tc.tile_pool(name="big", bufs=1))
        etp = ctx.enter_context(tc.tile_pool(name="etp", bufs=6))
        csb = ctx.enter_context(tc.tile_pool(name="csb", bufs=2))
        small = ctx.enter_context(tc.tile_pool(name="small", bufs=4))
        ypool = ctx.enter_context(tc.tile_pool(name="ypool", bufs=4))

        # ---- inputs: DMA order puts the step-0 critical path first ----
        xall = const.tile([128, NU, NC, TU], bf16)
        wq_sb = const.tile([128, NC, CF], bf16)
        wk_sb = const.tile([128, NC, CF], bf16)
        wv_sb = const.tile([128, NC, CF], bf16)
        bq_sb = const.tile([128, 1], f32)
        wo_sb = const.tile([128, F], bf16)

        def dma_x(j):
            nc.sync.dma_start(
                xall[:, j, :, :],
                xt_d[:, j * TU:(j + 1) * TU]
                .rearrange("(a p) t -> p a t", p=128))

        # unit 0 split into token-quarters so the first projection slices
        # start as soon as their quarter lands (cold DMA is ~3 us/MB)
        for q in range(4):
            nc.sync.dma_start(
                xall[:, 0, :, q * 128:(q + 1) * 128],
                xt_d[:, q * 128:(q + 1) * 128]
                .rearrange("(a p) t -> p a t", p=128))
        nc.sync.dma_start(wq_sb, wq_d.rearrange("(a p) n -> p a n", p=128))
        nc.sync.dma_start(wk_sb, wk_d.rearrange("(a p) n -> p a n", p=128))
        dma_x(1)
        nc.sync.dma_start(bq_sb, bq_d)
        nc.sync.dma_start(wv_sb, wv_d.rearrange("(a p) n -> p a n", p=128))
        dma_x(2)
        nc.sync.dma_start(wo_sb, wo_d)
        for j in range(3, NU):
            dma_x(j)

        ones_bf = const.tile([1, 64], bf16)
        nc.vector.memset(ones_bf, 1.0)

        # ---- persistent activations (all bf16) ----
        qt_sb = big.tile([128, T], bf16)       # [2 heads x 64 d, tokens]
        kt_sb = big.tile([128, T], bf16)
        # vaug per (b,kt,h): 128 cols = [V_h (64) | ones (1) | zeros (63)]
        # -> full-128-column stationary triggers FWL; ctx psum rows 65-127
        # are zeros and ignored.
        vaug_sb = big.tile([128, B, NKT, 2, 128], bf16)
        ctx2t_sb = big.tile([128, B, S], bf16)  # [2 heads x 64 d, b, tokens]

        nc.vector.memset(vaug_sb, 0.0)
        nc.vector.memset(vaug_sb[:, :, :, :, 64:65], 1.0)

        # ---- PSUM pools: 4 + 2 + 2 banks ----
        sc_ps = ctx.enter_context(
            tc.tile_pool(name="sc_ps", bufs=2, space="PSUM"))   # 4 banks
        pc_ps = ctx.enter_context(
            tc.tile_pool(name="pc_ps", bufs=2, space="PSUM"))   # 2 banks
        aux_ps = ctx.enter_context(
            tc.tile_pool(name="aux_ps", bufs=2, space="PSUM"))  # 2 banks

        # ---- projection half-unit slices (~1 us of PE each) ----
        def emit_q_slice(j, h):
            tq = j * TU + h * 256
            pq = aux_ps.tile([128, 256], f32, tag="aux", name=f"pq{j}_{h}")
            for c in range(NC):
                nc.tensor.matmul(pq, wq_sb[:, c, :],
                                 xall[:, j, c, h * 256:(h + 1) * 256],
                                 start=(c == 0), stop=(c == NC - 1))
            nc.vector.tensor_scalar_add(qt_sb[:, tq:tq + 256], pq, bq_sb)

        def emit_k_slice(j, h):
            tq = j * TU + h * 256
            pk = aux_ps.tile([128, 256], f32, tag="aux", name=f"pk{j}_{h}")
            for c in range(NC):
                nc.tensor.matmul(pk, wk_sb[:, c, :],
                                 xall[:, j, c, h * 256:(h + 1) * 256],
                                 start=(c == 0), stop=(c == NC - 1))
            nc.vector.tensor_copy(kt_sb[:, tq:tq + 256], pk)

        def emit_v_slice(j, h):
            pv = aux_ps.tile([128, 2, 128], f32, tag="aux",
                             name=f"pv{j}_{h}")
            for t in range(2):
                for c in range(NC):
                    nc.tensor.matmul(
                        pv[:, t, :],
                        xall[:, j, c,
                             h * 256 + t * 128:h * 256 + (t + 1) * 128],
                        wv_sb[:, c, :],
                        start=(c == 0), stop=(c == NC - 1))
            b = j // (NU // B)
            kt0 = (j % (NU // B)) * 4 + h * 2
            for t in range(2):
                nc.vector.tensor_copy(
                    vaug_sb[:, b, kt0 + t, 0, 0:64], pv[:, t, 0:64])
                nc.vector.tensor_copy(
                    vaug_sb[:, b, kt0 + t, 1, 0:64], pv[:, t, 64:128])

        # ---- attention steps: one per (b, qb, kt) ----
        steps = []
        for b in range(B):
            for qb in range(NQB):
                for kt in range(NKT):
                    steps.append((b, qb, kt))

        score_ps = {}
        exp_sb = {}
        ctx_ps = {}
        ctx_sb = {}
        norm_rcp = {}

        def emit_scores(i):
            b, qb, kt = steps[i]
            pss = sc_ps.tile([128, 2, 512], f32, tag="sc", name=f"sc{i}")
            for h in range(2):
                nc.tensor.matmul(
                    pss[:, h, :],
                    kt_sb[h * 64:(h + 1) * 64,
                          b * S + kt * 128:b * S + (kt + 1) * 128],
                    qt_sb[h * 64:(h + 1) * 64,
                          b * S + qb * 512:b * S + (qb + 1) * 512],
                    start=True, stop=True)
            score_ps[i] = pss

        def emit_exp(i):
            et = etp.tile([128, 2, 512], bf16, tag="et", name=f"et{i}")
            nc.scalar.activation(et, score_ps.pop(i), EXP)
            exp_sb[i] = et

        workq = deque()   # dynamic drain items: norm halves / outproj

        def emit_ctx(i):
            b, qb, kt = steps[i]
            if kt == 0:
                ctx_ps[(b, qb, 0)] = pc_ps.tile(
                    [128, 512], f32, tag="pc", name=f"pc{i}h0")
                ctx_ps[(b, qb, 1)] = pc_ps.tile(
                    [128, 512], f32, tag="pc", name=f"pc{i}h1")
            et = exp_sb.pop(i)
            for h in range(2):
                nc.tensor.matmul(
                    ctx_ps[(b, qb, h)],
                    vaug_sb[:, b, kt, h, :],
                    et[:, h, :],
                    start=(kt == 0), stop=(kt == NKT - 1))
            if kt == NKT - 1:
                # drain ctx psum to SBUF immediately: frees both pc bufs
                # before the next q-block's first ctx matmul needs them
                cs = csb.tile([65, 2, 512], bf16, tag="cs",
                              name=f"cs{b}{qb}")
                nc.vector.tensor_copy(
                    cs[:, 0, :], ctx_ps.pop((b, qb, 0))[0:65, :])
                nc.vector.tensor_copy(
                    cs[:, 1, :], ctx_ps.pop((b, qb, 1))[0:65, :])
                ctx_sb[(b, qb)] = cs
                workq.appendleft(("norm_dve", b, qb))

        def emit_norm_dve(b, qb):
            """DVE half: reciprocal of the bf16 rowsums."""
            cs = ctx_sb[(b, qb)]
            rs = small.tile([1, 2, 512], f32, tag="rs", name=f"rs{b}{qb}")
            nc.vector.tensor_copy(rs, cs[64:65, :, :])
            rcp = small.tile([1, 2, 512], f32, tag="rcp",
                             name=f"rcp{b}{qb}")
            nc.vector.reciprocal_approx_fast(rcp, rs)
            rcpb = small.tile([1, 2, 512], bf16, tag="rcpb",
                              name=f"rcpb{b}{qb}")
            nc.vector.tensor_copy(rcpb, rcp)
            norm_rcp[(b, qb)] = rcpb
            workq.append(("norm_pe", b, qb))

        def emit_norm_pe(b, qb):
            """PE half: broadcast matmuls + normalize into ctx2t."""
            cs = ctx_sb.pop((b, qb))
            rcpb = norm_rcp.pop((b, qb))
            dst = ctx2t_sb[:, b, qb * 512:(qb + 1) * 512]
            for h in range(2):
                pb = aux_ps.tile([64, 512], f32, tag="aux",
                                 name=f"pb{b}{qb}{h}")
                nc.tensor.matmul(pb, ones_bf, rcpb[0:1, h, :],
                                 start=True, stop=True)
                pbs = small.tile([64, 512], bf16, tag="pbs",
                                 name=f"pbs{b}{qb}{h}")
                nc.vector.tensor_copy(pbs, pb)
                nc.vector.tensor_mul(dst[h * 64:(h + 1) * 64, :],
                                     cs[0:64, h, :], pbs)
            for tt in range(4):
                workq.append(("op", b, qb, tt))

        def emit_outproj(b, qb, tt):
            tok0 = qb * 512 + tt * 128
            ysb = ypool.tile([128, 1024], bf16, tag="ysb",
                             name=f"ysb{b}{qb}{tt}")
            for wh in range(2):
                py = aux_ps.tile([128, 512], f32, tag="aux",
                                 name=f"py{b}{qb}{tt}{wh}")
                nc.tensor.matmul(
                    py, ctx2t_sb[:, b, tok0:tok0 + 128],
                    wo_sb[:, wh * 512:(wh + 1) * 512],
                    start=True, stop=True)
                nc.vector.tensor_copy(ysb[:, wh * 512:(wh + 1) * 512], py)
            nc.sync.dma_start(
                yp_d[b * S + tok0:b * S + tok0 + 128, :], ysb)

        # ---- background work queue: half-unit slices with deadlines ----
        bgq = []
        for u in range(1, NU):
            base = 0 if u < NU // B else 64
            uu = u % (NU // B)
            for h in range(2):
                kt0 = uu * 4 + h * 2
                bgq.append((base + kt0 - 2, "k", u, h))
                bgq.append((base + kt0 + CTX_LAG - 1, "v", u, h))
                bgq.append((base + 16 * uu - 4 + h, "q", u, h))
        bgq.sort(key=lambda t: t[0])
        bgq = deque(bgq)

        SLICE_COST = 1040
        WORK_COST = {"norm_dve": 60, "norm_pe": 860, "op": 900}
        BG_TOTAL = (len(bgq) * SLICE_COST + 8 * WORK_COST["norm_dve"]
                    + 8 * WORK_COST["norm_pe"] + 32 * WORK_COST["op"])
        RATE = BG_TOTAL / 124.0
        spent = [0.0]

        def emit_bg_slice():
            _, kind, j, h = bgq.popleft()
            if kind == "q":
                emit_q_slice(j, h)
            elif kind == "k":
                emit_k_slice(j, h)
            else:
                emit_v_slice(j, h)
            spent[0] += SLICE_COST

        def emit_work_item():
            item = workq.popleft()
            if item[0] == "norm_dve":
                emit_norm_dve(item[1], item[2])
            elif item[0] == "norm_pe":
                emit_norm_pe(item[1], item[2])
            else:
                emit_outproj(item[1], item[2], item[3])
            spent[0] += WORK_COST[item[0]]

        def drain_background(i, cap=1500):
            # norm_dve is DVE-only and unblocks the ctx psum chain: always
            while workq and workq[0][0] == "norm_dve":
                emit_work_item()
            # hard deadlines next, regardless of budget
            while bgq and bgq[0][0] <= i:
                emit_bg_slice()
            start = spent[0]
            target = (i + 1) * RATE
            while spent[0] < target and spent[0] - start < cap:
                if workq and len(workq) > 5:
                    emit_work_item()
                elif bgq:
                    emit_bg_slice()
                elif workq:
                    emit_work_item()
                else:
                    break

        # ---- prologue: minimal path to the first exp ----
        emit_k_slice(0, 0)
        emit_q_slice(0, 0)
        emit_q_slice(0, 1)
        emit_scores(0)
        emit_scores(1)
        emit_exp(0)
        emit_k_slice(0, 1)
        emit_v_slice(0, 0)
        emit_v_slice(0, 1)

        # ---- main loop ----
        for i in range(2, len(steps)):
            emit_scores(i)
            emit_exp(i - 1)
            if i >= CTX_LAG:
                emit_ctx(i - CTX_LAG)
            drain_background(i)
        emit_exp(len(steps) - 1)
        for i in range(len(steps) - CTX_LAG, len(steps)):
            emit_ctx(i)
        while workq or bgq:
            if bgq:
                emit_bg_slice()
            else:
                emit_work_item()

    nc.compile()
    return nc


_NC = None


def _to_bf16(a):
    import ml_dtypes
    return np.asarray(a, dtype=np.float32).astype(ml_dtypes.bfloat16)


def make_in_maps(inputs):
    """Build the 8 per-core input maps from full-precision inputs."""
    x = np.asarray(inputs["x"], dtype=np.float32)
    sc = 1.0 / np.sqrt(np.float32(DK))
    xT = np.ascontiguousarray(x.reshape(T, F).T)
    xT16 = _to_bf16(xT)
    in_maps = []
    for c in range(N_CORES):
        sl = slice(c * CF, (c + 1) * CF)
        in_maps.append({
            "xT": xT16,
            "Wq": _to_bf16(np.asarray(inputs["Wq"])[:, sl] * sc),
            "Wk": _to_bf16(np.asarray(inputs["Wk"])[:, sl]),
            "Wv": _to_bf16(np.asarray(inputs["Wv"])[:, sl]),
            "bq": np.ascontiguousarray(
                (np.asarray(inputs["bq"])[sl] * sc)
                .astype(np.float32).reshape(CF, 1)),
            "Wo": _to_bf16(np.asarray(inputs["Wo"])[sl, :]),
        })
    return in_maps


def combine_outputs(results, inputs):
    """Sum per-core bf16 partials, add host-side bias terms."""
    y = np.zeros((T, F), dtype=np.float64)
    for c in range(N_CORES):
        y += np.asarray(results[c]["yp"], dtype=np.float64)
    bo = np.asarray(inputs["bo"], dtype=np.float64)
    bv = np.asarray(inputs["bv"], dtype=np.float64)
    Wo = np.asarray(inputs["Wo"], dtype=np.float64)
    y += bo + bv @ Wo
    return y.astype(np.float32).reshape(B, S, F)


def kernel(x, Wq, bq, Wk, bk, Wv, bv, Wo, bo):
    global _NC
    if _NC is None:
        _NC = build_program()
    inputs = {"x": x, "Wq": Wq, "bq": bq, "Wk": Wk, "bk": bk,
              "Wv": Wv, "bv": bv, "Wo": Wo, "bo": bo}
    in_maps = make_in_maps(inputs)
    res = run_bass_kernel_spmd(_NC, in_maps, list(range(N_CORES)))
    return combine_outputs(res.results, inputs)
